# revision 1
# baseline (speedup 1.0000x reference)
"""Complex self-attention on 8 Trainium2 NeuronCores (Bass/Tile).

Reference computation (B=2, S=2048, F=1024, H=16, D=64):
    Q/K/V = complex_linear(x, W{q,k,v});  scores = Re(Q K^H) * D^-0.5
    attn = softmax(scores + mask_bias);  out = complex_linear(attn @ V, Wo)
    return stack([out_r, out_i])            # [2, B, S, F]

Sharding: 8 cores = 2 batches x 4 head-groups (4 heads each). Each core
computes its heads' Q/K/V projections, the attention, and a *partial*
output projection (contraction over its 256 features of Wo); the host
sums the 4 partials per batch and adds bo (the collective).

Complex arithmetic is folded into real matmuls by stacking (re, im)
parts along the contraction axis with host-prepped weight layouts:
    Xcat^T = [x_r^T ; x_i^T]   [2F, S]
    WQ[:, c<64]  = [Wq_r ; -Wq_i] col,  WQ[:, c>=64] = [Wq_i ; Wq_r] col
so one f32r matmul chain yields (Qr | Qi) per head, and head-local
tiles [128, *] carry (re 0:64, im 64:128) on the partition axis.

Softmax: no max subtraction (scaled scores have sigma~1.4, |s|<9, exp
is safe in f32); denominator comes from an appended mask column on the
V tiles, so the attn @ V matmul also yields sum_k exp * mask; division
happens per-partition in the natural [q, d] layout.
"""

import sys

if "/opt/trn_rl_repo" not in sys.path:
    sys.path.insert(0, "/opt/trn_rl_repo")

import numpy as np
import ml_dtypes

B, S, F = 2, 2048, 1024
H, D = 16, 64
NCORES = 8
HL = 4           # heads per core
D2 = 2 * D       # 128 = (re|im) feature rows per head
G = 2 * F // 128  # 16 contraction tiles over [x_r; x_i]
NST = S // 128    # 16 query/key 128-tiles
NQB = S // 512    # 4 query 512-blocks

BF16 = ml_dtypes.bfloat16

_CACHE = {}


def _build_program(with_bias=False):
    from concourse import bass, bacc, mybir, tile

    F32 = mybir.dt.float32
    F32R = mybir.dt.float32r
    BF = mybir.dt.bfloat16
    EXP = mybir.ActivationFunctionType.Exp

    nc = bacc.Bacc("TRN2", target_bir_lowering=False, debug=False)

    xcat = nc.dram_tensor("xcat", [2 * F, S], F32R, kind="ExternalInput")
    # Karatsuba weight blocks: [F, 3, HL*D] with m in {Wr, Wi, Wr+Wi}
    WK3 = 3 * HL * D
    wq_d = nc.dram_tensor("wq", [F, WK3], F32R, kind="ExternalInput")
    wk_d = nc.dram_tensor("wk", [F, WK3], F32R, kind="ExternalInput")
    wv_d = nc.dram_tensor("wv", [F, WK3], F32R, kind="ExternalInput")
    wor_d = nc.dram_tensor("wor", [HL * D2, F], BF, kind="ExternalInput")
    woi_d = nc.dram_tensor("woi", [HL * D2, F], BF, kind="ExternalInput")
    bq_d = nc.dram_tensor("bqrep", [128, HL * D2], BF, kind="ExternalInput")
    bk_d = nc.dram_tensor("bkrep", [128, HL * D2], BF, kind="ExternalInput")
    bv_d = nc.dram_tensor("bvrep", [128, HL * D2], BF, kind="ExternalInput")
    mask_d = nc.dram_tensor("maskcols", [128, NST], F32, kind="ExternalInput")
    ident_d = nc.dram_tensor("ident", [128, 128], F32R, kind="ExternalInput")
    out_r = nc.dram_tensor("out_r", [S, F], F32, kind="ExternalOutput")
    out_i = nc.dram_tensor("out_i", [S, F], F32, kind="ExternalOutput")

    scale = 1.0 / float(np.sqrt(D))

    with tile.TileContext(nc) as tc, nc.allow_low_precision("f32r/bf16 pipeline"):
        with (
            tc.tile_pool(name="consts", bufs=1) as cpool,
            tc.tile_pool(name="qkt", bufs=1) as qkt_pool,
            tc.tile_pool(name="vaug", bufs=1) as vaug_pool,
        ):
            ident = cpool.tile([128, 128], F32R)
            nc.sync.dma_start(ident[:], ident_d.ap())
            ident_bf = cpool.tile([128, 128], BF)
            nc.gpsimd.dma_start(ident_bf[:], ident_d.ap().bitcast(F32))
            mask_sb = cpool.tile([128, NST], F32)
            nc.sync.dma_start(mask_sb[:], mask_d.ap())
            bq_sb = cpool.tile([128, HL * D2], BF)
            nc.sync.dma_start(bq_sb[:], bq_d.ap())
            bk_sb = cpool.tile([128, HL * D2], BF)
            nc.sync.dma_start(bk_sb[:], bk_d.ap())
            bv_sb = cpool.tile([128, HL * D2], BF)
            nc.sync.dma_start(bv_sb[:], bv_d.ap())

            # Resident activations: transposed Q/K (f32r) and masked V (+mask col, bf16)
            qt = qkt_pool.tile([128, HL, S], F32R)   # [d_ri, h, s]
            kt = qkt_pool.tile([128, HL, S], F32R)
            va = vaug_pool.tile([128, HL, NST, D2 + 1], BF)  # [k, h, k_tile, d_ri|mask]

            # ---------------- Phase 1a: Q,K projections (Karatsuba), transpose
            # m1 = xr@Wr, m2 = xi@Wi, m3 = (xr+xi)@(Wr+Wi);
            # re = m1-m2, im = m3-m1-m2. 3 matmul chains instead of 4.
            # wv lives in its own pool so its prefetch DMAs (paced through the
            # 1a loop) land in SBUF that does not overlap the 1a pools.
            GF = F // 128  # 8 contraction tiles per m-chain
            HD = HL * D    # 256 columns per m-chain
            wvpool = tc.alloc_tile_pool(name="wv", bufs=1)
            wv_sb = wvpool.tile([128, GF, WK3], F32R)
            with (
                tc.tile_pool(name="wqk", bufs=1) as wpool,
                tc.tile_pool(name="xcol", bufs=3) as xpool,
                tc.tile_pool(name="xsum", bufs=2) as xsum_pool,
                tc.tile_pool(name="stage", bufs=2) as spool,
                tc.tile_pool(name="proj_ps", bufs=6, space="PSUM") as proj_ps,
                tc.tile_pool(name="tr_ps", bufs=2, space="PSUM") as tr_ps,
            ):
                wq_sb = wpool.tile([128, GF, WK3], F32R, tag="wq")
                wk_sb = wpool.tile([128, GF, WK3], F32R, tag="wk")
                WCH = 2  # g-tiles per weight-DMA chunk

                def load_w_chunk(w_sb, w_d, c):
                    nc.sync.dma_start(
                        w_sb[:, c : c + WCH, :],
                        w_d.ap()[c * 128 : (c + WCH) * 128, :].rearrange(
                            "(g p) n -> p g n", p=128
                        ),
                    )

                def proj_chains(ps_pool, xlo, xhi, xs, w_sb, tag):
                    m1 = ps_pool.tile([128, HD], F32, tag="pm", name=f"{tag}_m1")
                    m2 = ps_pool.tile([128, HD], F32, tag="pm", name=f"{tag}_m2")
                    m3 = ps_pool.tile([128, HD], F32, tag="pm", name=f"{tag}_m3")
                    for g in range(GF):
                        st_f = g == 0
                        sp_f = g == GF - 1
                        nc.tensor.matmul(m1[:], xlo[:, g, :], w_sb[:, g, 0:HD],
                                         start=st_f, stop=sp_f)
                        nc.tensor.matmul(m2[:], xhi[:, g, :], w_sb[:, g, HD : 2 * HD],
                                         start=st_f, stop=sp_f)
                        nc.tensor.matmul(m3[:], xs[:, g, :], w_sb[:, g, 2 * HD : 3 * HD],
                                         start=st_f, stop=sp_f)
                    return m1, m2, m3

                def hd_view(ap2d):
                    return ap2d.rearrange("p (h d) -> p h d", d=D)

                def combine_nat(m1, m2, m3, nat, tmp, c2, b_sb):
                    # nat[:, h*128+(0:64)] = m1-m2 ; nat[:, h*128+(64:128)] = m3-m1-m2
                    # (walrus: TT reads at most one PSUM operand -> stage m2)
                    natv = nat[:].rearrange("p (h c) -> p h c", c=D2)
                    nc.vector.tensor_copy(c2[:], m2[:])
                    nc.vector.tensor_sub(natv[:, :, 0:D], hd_view(m1[:]), hd_view(c2[:]))
                    nc.vector.tensor_sub(tmp[:], m3[:], c2[:])
                    nc.vector.tensor_sub(natv[:, :, D:D2], hd_view(tmp[:]), hd_view(m1[:]))
                    if with_bias:
                        nc.vector.tensor_add(nat[:], nat[:], b_sb[:])

                for st in range(NST):
                    xlo = xpool.tile([128, GF, 128], F32R, tag="xlo", name="xlo")
                    nc.sync.dma_start(
                        xlo[:],
                        xcat.ap()[0:F, st * 128 : (st + 1) * 128].rearrange(
                            "(g p) m -> p g m", p=128
                        ),
                    )
                    xhi = xpool.tile([128, GF, 128], F32R, tag="xhi", name="xhi")
                    nc.sync.dma_start(
                        xhi[:],
                        xcat.ap()[F : 2 * F, st * 128 : (st + 1) * 128].rearrange(
                            "(g p) m -> p g m", p=128
                        ),
                    )
                    if st == 0:
                        # grouped by tensor: PE consumes Q chains first, then K, V
                        for w_sb, w_d in ((wq_sb, wq_d), (wk_sb, wk_d), (wv_sb, wv_d)):
                            for c in range(0, GF, WCH):
                                load_w_chunk(w_sb, w_d, c)
                    xs = xsum_pool.tile([128, GF, 128], F32R, name="xs")
                    nc.vector.tensor_add(xs[:], xlo[:], xhi[:])

                    q_m = proj_chains(proj_ps, xlo, xhi, xs, wq_sb, "q")
                    qn = spool.tile([128, HL * D2], F32R, tag="nat", name="qn")
                    tmp = spool.tile([128, HD], F32, tag="tmp", name="tmpq")
                    c2q = spool.tile([128, HD], F32, tag="c2", name="c2q")
                    combine_nat(*q_m, qn, tmp, c2q, bq_sb)
                    k_m = proj_chains(proj_ps, xlo, xhi, xs, wk_sb, "k")
                    kn = spool.tile([128, HL * D2], F32R, tag="nat", name="kn")
                    tmp2 = spool.tile([128, HD], F32, tag="tmp", name="tmpk")
                    c2k = spool.tile([128, HD], F32, tag="c2", name="c2k")
                    combine_nat(*k_m, kn, tmp2, c2k, bk_sb)
                    for h in range(HL):
                        qtp = tr_ps.tile([128, 128], F32R, tag="tr")
                        nc.tensor.transpose(
                            qtp[:], qn[:, h * D2 : (h + 1) * D2], ident[:]
                        )
                        nc.vector.tensor_copy(
                            qt[:, h, st * 128 : (st + 1) * 128], qtp[:]
                        )
                        ktp = tr_ps.tile([128, 128], F32R, tag="tr")
                        nc.tensor.transpose(
                            ktp[:], kn[:, h * D2 : (h + 1) * D2], ident[:]
                        )
                        nc.vector.tensor_copy(
                            kt[:, h, st * 128 : (st + 1) * 128], ktp[:]
                        )
                    # V chains share the same psum slots (freed by Q combines)
                    v_m1, v_m2, v_m3 = proj_chains(proj_ps, xlo, xhi, xs, wv_sb, "v")
                    c2v = spool.tile([128, HD], F32, tag="c2", name="c2v")
                    nc.vector.tensor_copy(c2v[:], v_m2[:])
                    t_re = spool.tile([128, HD], F32, tag="vt", name="t_re")
                    nc.vector.tensor_sub(t_re[:], v_m1[:], c2v[:])
                    t_im = spool.tile([128, HD], F32, tag="vt", name="t_im")
                    nc.vector.tensor_sub(t_im[:], v_m3[:], c2v[:])
                    nc.vector.tensor_sub(t_im[:], t_im[:], v_m1[:])
                    if with_bias:
                        bv_v = bv_sb[:].rearrange("p (h c) -> p h c", c=D2)
                        t_re_v = t_re[:].rearrange("p (h d) -> p h d", d=D)
                        t_im_v = t_im[:].rearrange("p (h d) -> p h d", d=D)
                        nc.vector.tensor_add(t_re_v[:], t_re_v[:], bv_v[:, :, 0:D])
                        nc.vector.tensor_add(t_im_v[:], t_im_v[:], bv_v[:, :, D:D2])
                    COPY = mybir.ActivationFunctionType.Copy
                    for h in range(HL):
                        nc.vector.tensor_scalar_mul(
                            va[:, h, st, 0:D],
                            t_re[:, h * D : (h + 1) * D],
                            mask_sb[:, st : st + 1],
                        )
                        nc.vector.tensor_scalar_mul(
                            va[:, h, st, D:D2],
                            t_im[:, h * D : (h + 1) * D],
                            mask_sb[:, st : st + 1],
                        )
                        nc.vector.tensor_copy(
                            va[:, h, st, D2 : D2 + 1], mask_sb[:, st : st + 1]
                        )

            wvpool.release()

            # -------- Phase 2+3: attention (1024-wide q blocks) + O-proj ------
            # Loop order: q-block outer, head inner; after all heads of a
            # q-block finish, that block's output projection runs — its PE
            # matmuls fill the ACT-bound (exp) stretches of the next block.
            QW = 512  # q-block width
            with (
                tc.tile_pool(name="atp", bufs=1) as at_pool,
                tc.tile_pool(name="wo", bufs=1) as wopool,
                tc.tile_pool(name="p_sb", bufs=2 * NST + 2) as p_pool,
                tc.tile_pool(name="asb", bufs=4) as a_pool,
                tc.tile_pool(name="rcp", bufs=4) as r_pool,
                tc.tile_pool(name="ost", bufs=4) as opool,
                tc.tile_pool(name="sc_ps", bufs=4, space="PSUM") as sc_ps,
                tc.tile_pool(name="a_ps", bufs=2, space="PSUM") as a_ps,
                tc.tile_pool(name="o_ps", bufs=2, space="PSUM") as o_ps,
            ):
                at = at_pool.tile([128, HL, S], BF)  # [d_ri, h, s] attn out^T
                wor_sb = wopool.tile([128, HL, F], BF, tag="wor")
                nc.sync.dma_start(
                    wor_sb[:], wor_d.ap().rearrange("(h p) n -> p h n", p=128)
                )
                woi_sb = wopool.tile([128, HL, F], BF, tag="woi")
                nc.sync.dma_start(
                    woi_sb[:], woi_d.ap().rearrange("(h p) n -> p h n", p=128)
                )
                def oproj_block(st):
                    # output projection for s rows [st*128, (st+1)*128)
                    for fo in range(2):
                        opr = o_ps.tile([128, 512], F32, tag="o")
                        opi = o_ps.tile([128, 512], F32, tag="o")
                        for h2 in range(HL):
                            lhsT = at[:, h2, st * 128 : (st + 1) * 128]
                            nc.tensor.matmul(
                                opr[:], lhsT, wor_sb[:, h2, fo * 512 : (fo + 1) * 512],
                                start=(h2 == 0), stop=(h2 == HL - 1),
                            )
                            nc.tensor.matmul(
                                opi[:], lhsT, woi_sb[:, h2, fo * 512 : (fo + 1) * 512],
                                start=(h2 == 0), stop=(h2 == HL - 1),
                            )
                        for ops, dram in ((opr, out_r), (opi, out_i)):
                            osb = opool.tile([128, 512], F32, tag="ost")
                            nc.vector.tensor_copy(osb[:], ops[:])
                            nc.sync.dma_start(
                                dram.ap()[
                                    st * 128 : (st + 1) * 128,
                                    fo * 512 : (fo + 1) * 512,
                                ],
                                osb[:],
                            )

                NBLK = QW // 128
                for qbb in range(S // QW):
                    q0 = qbb * QW
                    for h in range(HL):
                        if qbb > 0:
                            oproj_block((qbb - 1) * NBLK + h)
                        p_tiles = []
                        for ktile in range(NST):
                            sps = sc_ps.tile([128, QW], F32, tag="sc")
                            for half in range(QW // 512):
                                nc.tensor.matmul(
                                    sps[:, half * 512 : (half + 1) * 512],
                                    kt[:, h, ktile * 128 : (ktile + 1) * 128],
                                    qt[:, h, q0 + half * 512 : q0 + (half + 1) * 512],
                                )
                            pt = p_pool.tile([128, QW], BF, tag="p")
                            nc.scalar.activation(pt[:], sps[:], EXP, scale=scale)
                            p_tiles.append(pt)
                        for qs in range(QW // 128):
                            aps = a_ps.tile([128, D2 + 1], F32, tag="a")
                            for ktile in range(NST):
                                nc.tensor.matmul(
                                    aps[:],
                                    p_tiles[ktile][:, qs * 128 : (qs + 1) * 128],
                                    va[:, h, ktile, :],
                                    start=(ktile == 0), stop=(ktile == NST - 1),
                                )
                            rcp = r_pool.tile([128, 1], F32, tag="r")
                            nc.vector.reciprocal(rcp[:], aps[:, D2 : D2 + 1])
                            asb = a_pool.tile([128, D2], BF, tag="asb")
                            nc.vector.tensor_scalar_mul(asb[:], aps[:, 0:D2], rcp[:])
                            nc.sync.dma_start(
                                at[:, h, q0 + qs * 128 : q0 + (qs + 1) * 128],
                                asb[:],
                                transpose=True,
                            )
                # last q-block's output projection
                if qbb == (S // QW) - 1:
                    for st in range(qbb * NBLK, (qbb + 1) * NBLK):
                        oproj_block(st)

    nc.compile()
    return nc


def _get_program(with_bias=False):
    key = f"nc_bias{with_bias}"
    if key not in _CACHE:
        _CACHE[key] = _build_program(with_bias=with_bias)
    return _CACHE[key]


def _prep_core_inputs(inputs, core):
    """Host-side shard prep for one core (batch b, heads h0..h0+3)."""
    f32 = np.float32
    b = core // (NCORES // B)
    h0 = (core % (NCORES // B)) * HL
    hs = slice(h0 * D, (h0 + HL) * D)  # feature slice of this core's heads

    xr = np.asarray(inputs["x_r"][b], dtype=f32)
    xi = np.asarray(inputs["x_i"][b], dtype=f32)
    xcat = np.concatenate([xr.T, xi.T], axis=0)  # [2F, S]
    xcat = np.ascontiguousarray(xcat)

    def wstack(wr, wi):
        # Karatsuba blocks [F, 3, HL*D]: m0 = Wr, m1 = Wi, m2 = Wr+Wi
        wr = np.asarray(wr, dtype=f32)[:, hs]
        wi = np.asarray(wi, dtype=f32)[:, hs]
        w = np.stack([wr, wi, wr + wi], axis=1)  # [F, 3, HL*D]
        return np.ascontiguousarray(w.reshape(F, 3 * HL * D))

    def brep(br, bi):
        br = np.asarray(br, dtype=f32)[hs].reshape(HL, D)
        bi = np.asarray(bi, dtype=f32)[hs].reshape(HL, D)
        bcat = np.concatenate([br, bi], axis=1).reshape(HL * D2)
        return np.ascontiguousarray(
            np.broadcast_to(bcat, (128, HL * D2)).astype(BF16)
        )

    def wostack(wor, woi):
        # rows r<64 -> wo_top[d], r>=64 -> wo_bot[d]  per head
        wor = np.asarray(wor, dtype=f32)[hs].reshape(HL, D, F)
        woi = np.asarray(woi, dtype=f32)[hs].reshape(HL, D, F)
        w = np.empty((HL, D2, F), dtype=f32)
        w[:, :D] = wor
        w[:, D:] = woi
        return np.ascontiguousarray(w.reshape(HL * D2, F).astype(BF16))

    mask = np.asarray(inputs["mask"][b], dtype=f32)
    mask_cols = np.ascontiguousarray(mask.reshape(NST, 128).T)

    return {
        "xcat": xcat,
        "wq": wstack(inputs["Wq_r"], inputs["Wq_i"]),
        "wk": wstack(inputs["Wk_r"], inputs["Wk_i"]),
        "wv": wstack(inputs["Wv_r"], inputs["Wv_i"]),
        "wor": wostack(inputs["Wo_r"], -np.asarray(inputs["Wo_i"], dtype=f32)),
        "woi": wostack(inputs["Wo_i"], inputs["Wo_r"]),
        "bqrep": brep(inputs["bq_r"], inputs["bq_i"]),
        "bkrep": brep(inputs["bk_r"], inputs["bk_i"]),
        "bvrep": brep(inputs["bv_r"], inputs["bv_i"]),
        "maskcols": mask_cols,
        "ident": np.eye(128, dtype=f32),
    }


def kernel(_trace=False, _trace_kwargs=None, **inputs):
    from concourse.bass_utils import run_bass_kernel_spmd

    with_bias = any(
        np.any(np.asarray(inputs[k]))
        for k in ("bq_r", "bq_i", "bk_r", "bk_i", "bv_r", "bv_i")
    )
    nc = _get_program(with_bias=bool(with_bias))
    in_maps = [_prep_core_inputs(inputs, c) for c in range(NCORES)]
    res = run_bass_kernel_spmd(
        nc, in_maps, core_ids=list(range(NCORES)),
        trace=_trace, **(_trace_kwargs or {}),
    )
    _CACHE["last_results"] = res

    bo_r = np.asarray(inputs["bo_r"], dtype=np.float32)
    bo_i = np.asarray(inputs["bo_i"], dtype=np.float32)
    out = np.empty((2, B, S, F), dtype=np.float32)
    cpb = NCORES // B
    for b in range(B):
        cores = range(b * cpb, (b + 1) * cpb)
        out[0, b] = sum(res.results[c]["out_r"] for c in cores) + bo_r
        out[1, b] = sum(res.results[c]["out_i"] for c in cores) + bo_i
    return out



# revision 46
# speedup vs baseline: 1.1952x; 1.1952x over previous
"""Complex self-attention on 8 Trainium2 NeuronCores (Bass/Tile).

Reference computation (B=2, S=2048, F=1024, H=16, D=64):
    Q/K/V = complex_linear(x, W{q,k,v});  scores = Re(Q K^H) * D^-0.5
    attn = softmax(scores + mask_bias);  out = complex_linear(attn @ V, Wo)
    return stack([out_r, out_i])            # [2, B, S, F]

Sharding: 8 cores = 2 batches x 4 head-groups (4 heads each). Each core
computes its heads' Q/K/V projections, the attention, and a *partial*
output projection (contraction over its 256 features of Wo); the host
sums the 4 partials per batch and adds bo (the collective).

Phase 1 (projections) runs on the PE in fp8e4 DoubleRow perf mode with
compensated (hi+lo) operands: X ~ Xhi+Xlo, W ~ Whi+Wlo, and
  X@W ~ Xhi@Whi + (Xhi@Wlo + Xlo@Whi)
The hi.hi products pair across adjacent 128-row contraction chunks in
one DoubleRow matmul; the two cross products of one chunk pair in
another.  12 DR instructions replace 16 bf16-equivalent matmuls per
256-col chain, at ~bf16 accuracy (residual ~0.1%).  Karatsuba over the
complex structure (m1=xr@Wr, m2=xi@Wi, m3=(xr+xi)@(Wr+Wi)) cuts 4 real
products to 3.  Scale folding: X is scaled by 16, W by 64 on the host
(fp8e4 max finite = 240); the product scale PS=1024 is divided out of
the exp scale (2^-23) and the Wo weights.

Softmax: no max subtraction (scaled scores have sigma~1.4, |s|<10, exp
is safe); denominator comes from an appended mask column on the V
tiles, so attn @ V also yields sum_k exp * mask; division happens
per-partition in the natural [q, d] layout.

Engine split: PE matmuls; DVE q-combines + va writes + rcp; Pool
(gpsimd) k/v-combines, attn normalize, oproj PSUM->SBUF copies; ACT
exp over 1024-wide ktile pairs; q/k transposes via DMA-transpose on the
two HWDGE queues (SP + ACT).
"""

import sys

if "/opt/trn_rl_repo" not in sys.path:
    sys.path.insert(0, "/opt/trn_rl_repo")

import numpy as np
import ml_dtypes

B, S, F = 2, 2048, 1024
H, D = 16, 64
NCORES = 8
HL = 4           # heads per core
D2 = 2 * D       # 128 = (re|im) feature rows per head
GF = F // 128    # 8 contraction chunks per m-chain
HD = HL * D      # 256 columns per m-chain
NST = S // 128   # 16 query/key 128-tiles
QW = 512         # q-block width in phase 2

SCX = 16.0       # host scale on x (and xr+xi)
SCW = 64.0       # host scale on W blocks
PS = SCX * SCW   # product scale carried by Q/K/V
FP8MAX = 230.0   # safety clip below fp8e4 max finite (240)

BF16 = ml_dtypes.bfloat16
FP8T = ml_dtypes.float8_e4m3

_CACHE = {}


def _build_program(with_bias=False):
    from concourse import bass, bacc, mybir, tile

    F32 = mybir.dt.float32
    BF = mybir.dt.bfloat16
    FP8 = mybir.dt.float8e4
    DR = mybir.MatmulPerfMode.DoubleRow
    EXP = mybir.ActivationFunctionType.Exp

    nc = bacc.Bacc("TRN2", target_bir_lowering=False, debug=False)

    # x planes [128p, st, (re|im|re+im), g, (hi,lo), 128s]; per (p,st): 6KB
    x3_d = nc.dram_tensor("x3", [128, NST, 3, GF, 2, 128], FP8, kind="ExternalInput")
    # weights [128p, g, (lo,hi), 3*HD]; 3 Karatsuba blocks (Wr | Wi | Wr+Wi)
    WK3 = 3 * HD
    wq_d = nc.dram_tensor("wq", [128, GF, 2, WK3], FP8, kind="ExternalInput")
    wk_d = nc.dram_tensor("wk", [128, GF, 2, WK3], FP8, kind="ExternalInput")
    wv_d = nc.dram_tensor("wv", [128, GF, 2, WK3], FP8, kind="ExternalInput")
    wor_d = nc.dram_tensor("wor", [HL * D2, F], BF, kind="ExternalInput")
    woi_d = nc.dram_tensor("woi", [HL * D2, F], BF, kind="ExternalInput")
    bq_d = nc.dram_tensor("bqrep", [128, HL * D2], BF, kind="ExternalInput")
    bk_d = nc.dram_tensor("bkrep", [128, HL * D2], BF, kind="ExternalInput")
    bv_d = nc.dram_tensor("bvrep", [128, HL * D2], BF, kind="ExternalInput")
    mask_d = nc.dram_tensor("maskcols", [128, NST], F32, kind="ExternalInput")
    identb_d = nc.dram_tensor("identb", [128, 128], BF, kind="ExternalInput")
    out_r = nc.dram_tensor("out_r", [S, F], F32, kind="ExternalOutput")
    out_i = nc.dram_tensor("out_i", [S, F], F32, kind="ExternalOutput")

    # exp(scores_psum * EXPSCALE) = exp(QK / sqrt(D)) with QK carrying PS^2
    expscale = 1.0 / (float(np.sqrt(D)) * PS * PS)

    with tile.TileContext(nc) as tc, nc.allow_low_precision("fp8/bf16 pipeline"):
        with (
            tc.tile_pool(name="consts", bufs=1) as cpool,
            tc.tile_pool(name="qkt", bufs=1) as qkt_pool,
            tc.tile_pool(name="vaug", bufs=1) as vaug_pool,
        ):
            mask_sb = cpool.tile([128, NST], F32)
            ident_bf = cpool.tile([128, 128], BF)
            bq_sb = cpool.tile([128, HL * D2], BF)
            bk_sb = cpool.tile([128, HL * D2], BF)
            bv_sb = cpool.tile([128, HL * D2], BF)

            # Resident activations: transposed Q/K (bf16) and masked V (+mask col)
            qt = qkt_pool.tile([128, HL, S], BF)   # [d_ri, h, s]
            kt = qkt_pool.tile([128, HL, S], BF)
            va = vaug_pool.tile([128, HL, NST, D2 + 1], BF)  # [k, h, ktile, d_ri|mask]

            # ---------------- Phase 1: Q,K,V projections (Karatsuba, fp8 DR)
            with (
                tc.tile_pool(name="wqkv", bufs=1) as wpool,
                tc.tile_pool(name="xcol", bufs=5) as xpool,
                tc.tile_pool(name="stage", bufs=2) as spool,
                tc.tile_pool(name="proj_ps", bufs=6, space="PSUM") as proj_ps,
                tc.tile_pool(name="tr_ps", bufs=2, space="PSUM") as tr_ps,
            ):
                # each weight tensor split in two tiles (g 0..3 | 4..7), the
                # halves DMAd on the two HWDGE queues in parallel
                GH = GF // 2
                wsb = {}
                for nm, d in (("wq", wq_d), ("wk", wk_d), ("wv", wv_d)):
                    w0 = wpool.tile([128, GH, 2, WK3], FP8, tag=f"{nm}0",
                                    name=f"{nm}0")
                    w1 = wpool.tile([128, GH, 2, WK3], FP8, tag=f"{nm}1",
                                    name=f"{nm}1")
                    wsb[nm] = (w0, w1)

                def load_w(nm, d):
                    t0, t1 = wsb[nm]
                    nc.sync.dma_start(t0[:], d.ap()[:, 0:GH])
                    nc.scalar.dma_start(t1[:], d.ap()[:, GH:GF])

                def load_x(st):
                    eng = nc.sync if st % 2 == 0 else nc.scalar
                    t = xpool.tile([128, 3, GF, 2, 128], FP8, tag="x", name=f"x{st}")
                    eng.dma_start(t[:], x3_d.ap()[:, st])
                    return t

                # queue order: wq | x0..x3 | wk | consts | wv  (DMA transfers
                # are serial; order matches first-use order)
                load_w("wq", wq_d)
                nc.scalar.dma_start(ident_bf[:], identb_d.ap())
                xts = {st: load_x(st) for st in range(4)}
                load_w("wk", wk_d)
                nc.scalar.dma_start(mask_sb[:], mask_d.ap())
                load_w("wv", wv_d)
                nc.scalar.dma_start(bq_sb[:], bq_d.ap())
                nc.scalar.dma_start(bk_sb[:], bk_d.ap())
                nc.scalar.dma_start(bv_sb[:], bv_d.ap())

                def dr_chain(ps, xt, v, w_pair, j):
                    c0, c1 = j * HD, (j + 1) * HD
                    for gp in range(0, GF, 2):
                        nc.tensor.matmul(
                            ps[:], xt[:, v, gp : gp + 2, 0, :],
                            w_pair[gp // GH][:, (gp % GH) : (gp % GH) + 2, 1, c0:c1],
                            start=(gp == 0), stop=False, perf_mode=DR,
                        )
                    for g in range(GF):
                        nc.tensor.matmul(
                            ps[:], xt[:, v, g, :, :],
                            w_pair[g // GH][:, g % GH, :, c0:c1],
                            start=False, stop=(g == GF - 1), perf_mode=DR,
                        )

                def proj_chains(xt, w_pair, tag):
                    ms = []
                    for j in range(3):
                        m = proj_ps.tile([128, HD], F32, tag="pm", name=f"{tag}_m{j}")
                        dr_chain(m, xt, j, w_pair, j)
                        ms.append(m)
                    return ms

                def hd_view(ap2d):
                    return ap2d.rearrange("p (h d) -> p h d", d=D)

                def combine_nat(m1, m2, m3, nat, tmp, c2, b_sb):
                    # nat[:, h*128+(0:64)] = m1-m2 ; nat[:, h*128+(64:128)] = m3-m1-m2
                    # (walrus: TT reads at most one PSUM operand -> stage m2;
                    # the stage copy runs on ACT, idle during phase 1)
                    natv = nat[:].rearrange("p (h c) -> p h c", c=D2)
                    nc.scalar.copy(c2[:], m2[:])
                    nc.vector.tensor_sub(natv[:, :, 0:D], hd_view(m1[:]), hd_view(c2[:]))
                    nc.vector.tensor_sub(tmp[:], m3[:], c2[:])
                    nc.vector.tensor_sub(natv[:, :, D:D2], hd_view(tmp[:]), hd_view(m1[:]))
                    if with_bias:
                        nc.vector.tensor_add(nat[:], nat[:], b_sb[:])

                ones4 = cpool.tile([128, HL], F32)
                nc.vector.memset(ones4[:], 1.0)

                def transpose_out(nat, dst, st):
                    # 4 head transposes into one psum tile, single copy out
                    tp = tr_ps.tile([128, HL, 128], BF, tag="tr")
                    for h in range(HL):
                        nc.tensor.transpose(
                            tp[:, h, :], nat[:, h * D2 : (h + 1) * D2], ident_bf[:]
                        )
                    nc.vector.tensor_copy(
                        dst[:, :, st * 128 : (st + 1) * 128], tp[:]
                    )

                def qwork(st, xt):
                    ms = proj_chains(xt, wsb["wq"], "q")
                    qn = spool.tile([128, HL * D2], BF, tag="qn", name="qn")
                    tmp = spool.tile([128, HD], F32, tag="tmp", name="tmpq")
                    c2 = spool.tile([128, HD], F32, tag="c2", name="c2q")
                    combine_nat(*ms, qn, tmp, c2, bq_sb)
                    return qn

                def kwork(st, xt):
                    ms = proj_chains(xt, wsb["wk"], "k")
                    kn = spool.tile([128, HL * D2], BF, tag="kn", name="kn")
                    tmp = spool.tile([128, HD], F32, tag="tmp", name="tmpk")
                    c2 = spool.tile([128, HD], F32, tag="c2", name="c2k")
                    combine_nat(*ms, kn, tmp, c2, bk_sb)
                    return kn

                def vwork(st, xt):
                    m1, m2, m3 = proj_chains(xt, wsb["wv"], "v")
                    c2 = spool.tile([128, HD], F32, tag="c2", name="c2v")
                    nc.scalar.copy(c2[:], m2[:])
                    t_re = spool.tile([128, HD], F32, tag="vt", name="t_re")
                    nc.vector.tensor_sub(t_re[:], m1[:], c2[:])
                    t_im = spool.tile([128, HD], F32, tag="vt", name="t_im")
                    nc.vector.tensor_sub(t_im[:], m3[:], c2[:])
                    nc.vector.tensor_sub(t_im[:], t_im[:], m1[:])
                    if with_bias:
                        bv_v = bv_sb[:].rearrange("p (h c) -> p h c", c=D2)
                        t_re_v = t_re[:].rearrange("p (h d) -> p h d", d=D)
                        t_im_v = t_im[:].rearrange("p (h d) -> p h d", d=D)
                        nc.vector.tensor_add(t_re_v[:], t_re_v[:], bv_v[:, :, 0:D])
                        nc.vector.tensor_add(t_im_v[:], t_im_v[:], bv_v[:, :, D:D2])
                    # va writes are SBUF-only -> Pool engine
                    nc.gpsimd.tensor_scalar_mul(
                        va[:, :, st, 0:D],
                        t_re[:].rearrange("p (h d) -> p h d", d=D),
                        mask_sb[:, st : st + 1],
                    )
                    nc.gpsimd.tensor_scalar_mul(
                        va[:, :, st, D:D2],
                        t_im[:].rearrange("p (h d) -> p h d", d=D),
                        mask_sb[:, st : st + 1],
                    )
                    nc.gpsimd.tensor_scalar_mul(
                        va[:, :, st, D2 : D2 + 1],
                        ones4[:].rearrange("p (h c) -> p h c", c=1),
                        mask_sb[:, st : st + 1],
                    )

                # ramp: tensor-major over st 0..3 (Q while wk/wv stream in);
                # a tile's transposes run one step later so combines (DVE for
                # q, Pool for k) never stall the PE
                pend = []  # (nat, dst, st) transposes to emit

                def flush_transposes():
                    for nat, dst, st in pend:
                        transpose_out(nat, dst, st)
                    pend.clear()

                qns = {st: qwork(st, xts[st]) for st in range(4)}
                for st in range(4):
                    pend.append((qns[st], qt, st))
                    kn = kwork(st, xts[st])
                    flush_transposes()
                    pend.append((kn, kt, st))
                for st in range(4):
                    vwork(st, xts[st])
                    flush_transposes()
                    xts[st + 4] = load_x(st + 4)
                for st in range(4, NST):
                    if st + 4 < NST:
                        xts[st + 4] = load_x(st + 4)
                    qn = qwork(st, xts[st])
                    flush_transposes()
                    pend.append((qn, qt, st))
                    kn = kwork(st, xts[st])
                    vwork(st, xts[st])
                    pend.append((kn, kt, st))
                    del xts[st]
                flush_transposes()

            # -------- Phase 2: attention (512-wide q blocks) + O-proj ------
            with (
                tc.tile_pool(name="atp", bufs=1) as at_pool,
                tc.tile_pool(name="wo", bufs=1) as wopool,
                tc.tile_pool(name="p_sb", bufs=10) as p_pool,
                tc.tile_pool(name="asb", bufs=4) as a_pool,
                tc.tile_pool(name="rcp", bufs=4) as r_pool,
                tc.tile_pool(name="ost", bufs=6) as opool,
                tc.tile_pool(name="sc_ps", bufs=2, space="PSUM") as sc_ps,
                tc.tile_pool(name="a_ps", bufs=2, space="PSUM") as a_ps,
                tc.tile_pool(name="o_ps", bufs=2, space="PSUM") as o_ps,
            ):
                at = at_pool.tile([128, HL, S], BF)  # [d_ri, h, s] attn out^T
                wor_sb = wopool.tile([128, HL, F], BF, tag="wor")
                nc.sync.dma_start(
                    wor_sb[:], wor_d.ap().rearrange("(h p) n -> p h n", p=128)
                )
                woi_sb = wopool.tile([128, HL, F], BF, tag="woi")
                nc.sync.dma_start(
                    woi_sb[:], woi_d.ap().rearrange("(h p) n -> p h n", p=128)
                )

                pend_osb = []

                def oproj_half(st, fo):
                    # output projection for s rows [st*128, (st+1)*128),
                    # f cols [fo*512, (fo+1)*512)
                    opr = o_ps.tile([128, 512], F32, tag="o")
                    opi = o_ps.tile([128, 512], F32, tag="o")
                    # r chain fully before i chain: the r drain (psum->sbuf
                    # copy) overlaps the i matmuls, freeing o_ps sooner
                    for ops, wsb2 in ((opr, wor_sb), (opi, woi_sb)):
                        for h2 in range(HL):
                            nc.tensor.matmul(
                                ops[:],
                                at[:, h2, st * 128 : (st + 1) * 128],
                                wsb2[:, h2, fo * 512 : (fo + 1) * 512],
                                start=(h2 == 0), stop=(h2 == HL - 1),
                            )
                    for ops, dram in ((opr, out_r), (opi, out_i)):
                        osb = opool.tile([128, 512], F32, tag="ost")
                        nc.vector.tensor_copy(osb[:], ops[:])
                        nc.sync.dma_start(
                            dram.ap()[
                                st * 128 : (st + 1) * 128,
                                fo * 512 : (fo + 1) * 512,
                            ],
                            osb[:],
                        )

                def attn_finish(h, q0, qs, aps):
                    rcp = r_pool.tile([128, 1], F32, tag="r")
                    nc.vector.reciprocal(rcp[:], aps[:, D2 : D2 + 1])
                    asb = a_pool.tile([128, D2], BF, tag="asb")
                    nc.vector.tensor_scalar_mul(asb[:], aps[:, 0:D2], rcp[:])
                    nc.sync.dma_start(
                        at[:, h, q0 + qs * 128 : q0 + (qs + 1) * 128],
                        asb[:],
                        transpose=True,
                    )

                NBLK = QW // 128
                for qbb in range(S // QW):
                    q0 = qbb * QW
                    for h in range(HL):
                        # scores + exp in 1024-wide ktile pairs
                        pts = []
                        for j in range(NST // 2):
                            sps = sc_ps.tile([128, 2, 512], F32, tag="sc")
                            for l in range(2):
                                k = 2 * j + l
                                nc.tensor.matmul(
                                    sps[:, l, :],
                                    kt[:, h, k * 128 : (k + 1) * 128],
                                    qt[:, h, q0 : q0 + QW],
                                )
                            pt = p_pool.tile([128, 2, 512], BF, tag="p")
                            nc.scalar.activation(pt[:], sps[:], EXP, scale=expscale)
                            pts.append(pt)
                            if j == 1 and qbb > 0:
                                oproj_half((qbb - 1) * NBLK + h, 0)
                                oproj_half((qbb - 1) * NBLK + h, 1)

                        def av(qs, klo, khi, aps):
                            for k in range(klo, khi):
                                nc.tensor.matmul(
                                    aps[:],
                                    pts[k // 2][:, k % 2, qs * 128 : (qs + 1) * 128],
                                    va[:, h, k, :],
                                    start=(k == 0), stop=(k == NST - 1),
                                )

                        # qs 0,1: split halves to hide the exp of pairs 4..7
                        last = qbb == S // QW - 1 and h == HL - 1
                        ap0 = a_ps.tile([128, D2 + 1], F32, tag="a")
                        ap1 = a_ps.tile([128, D2 + 1], F32, tag="a")
                        av(0, 0, 8, ap0)
                        av(1, 0, 8, ap1)
                        av(0, 8, NST, ap0)
                        attn_finish(h, q0, 0, ap0)
                        av(1, 8, NST, ap1)
                        attn_finish(h, q0, 1, ap1)
                        for qs in (2, 3):
                            aps = a_ps.tile([128, D2 + 1], F32, tag="a")
                            av(qs, 0, NST, aps)
                            if last and qs == 3:
                                # fill the at-transpose latency of qs2
                                oproj_half(qbb * NBLK + 0, 0)
                            attn_finish(h, q0, qs, aps)
                        if last:
                            # st-interleaved halves: each half's osb drain
                            # overlaps the next st's matmuls
                            for st2, fo in ((1, 0), (0, 1), (2, 0), (1, 1),
                                            (3, 0), (2, 1), (3, 1)):
                                oproj_half(qbb * NBLK + st2, fo)

    nc.compile()
    return nc


def _get_program(with_bias=False):
    key = f"nc_bias{with_bias}"
    if key not in _CACHE:
        _CACHE[key] = _build_program(with_bias=with_bias)
    return _CACHE[key]


def _split8(a):
    """f32 array -> (hi, lo) fp8e4 pair with hi+lo ~ a."""
    hi = a.astype(FP8T)
    lo = (a - hi.astype(np.float32)).astype(FP8T)
    return hi, lo


def _prep_x(inputs, b):
    f32 = np.float32
    xr = np.asarray(inputs["x_r"][b], dtype=f32).T * f32(SCX)  # [F, S]
    xi = np.asarray(inputs["x_i"][b], dtype=f32).T * f32(SCX)
    out = np.empty((128, NST, 3, GF, 2, 128), dtype=FP8T)
    for v, plane in enumerate((xr, xi, xr + xi)):
        assert np.abs(plane).max() < FP8MAX, "fp8 overflow in x plane"
        hi, lo = _split8(plane)
        for l, a in enumerate((hi, lo)):
            # [F, S] -> [128p, NST, GF, 128s]
            out[:, :, v, :, l, :] = a.reshape(GF, 128, NST, 128).transpose(
                1, 2, 0, 3
            )
    return {"x3": np.ascontiguousarray(out)}


def _prep_core_inputs(inputs, core, xmaps):
    """Host-side shard prep for one core (batch b, heads h0..h0+3)."""
    f32 = np.float32
    b = core // (NCORES // B)
    h0 = (core % (NCORES // B)) * HL
    hs = slice(h0 * D, (h0 + HL) * D)  # feature slice of this core's heads

    def wstack(wr, wi):
        # Karatsuba blocks [F, 3*HD] = (Wr | Wi | Wr+Wi), scaled, hi/lo fp8
        wr = np.asarray(wr, dtype=f32)[:, hs] * f32(SCW)
        wi = np.asarray(wi, dtype=f32)[:, hs] * f32(SCW)
        blk = np.concatenate([wr, wi, wr + wi], axis=1)  # [F, 3*HD]
        assert np.abs(blk).max() < FP8MAX, "fp8 overflow in weights"
        hi, lo = _split8(blk)

        def re3(a):  # [F, 3HD] -> [128, GF, 3HD]
            return a.reshape(GF, 128, 3 * HD).transpose(1, 0, 2)

        out = np.empty((128, GF, 2, 3 * HD), dtype=FP8T)
        out[:, :, 0, :] = re3(lo)   # (lo, hi) order pairs with x's (hi, lo)
        out[:, :, 1, :] = re3(hi)
        return np.ascontiguousarray(out)

    def brep(br, bi):
        br = np.asarray(br, dtype=f32)[hs].reshape(HL, D) * f32(PS)
        bi = np.asarray(bi, dtype=f32)[hs].reshape(HL, D) * f32(PS)
        bcat = np.concatenate([br, bi], axis=1).reshape(HL * D2)
        return np.ascontiguousarray(
            np.broadcast_to(bcat, (128, HL * D2)).astype(BF16)
        )

    def wostack(wor, woi):
        # rows r<64 -> wo_top[d], r>=64 -> wo_bot[d]  per head; 1/PS folded in
        wor = np.asarray(wor, dtype=f32)[hs].reshape(HL, D, F) / f32(PS)
        woi = np.asarray(woi, dtype=f32)[hs].reshape(HL, D, F) / f32(PS)
        w = np.empty((HL, D2, F), dtype=f32)
        w[:, :D] = wor
        w[:, D:] = woi
        return np.ascontiguousarray(w.reshape(HL * D2, F).astype(BF16))

    mask = np.asarray(inputs["mask"][b], dtype=f32)
    mask_cols = np.ascontiguousarray(mask.reshape(NST, 128).T)

    m = {
        "wq": wstack(inputs["Wq_r"], inputs["Wq_i"]),
        "wk": wstack(inputs["Wk_r"], inputs["Wk_i"]),
        "wv": wstack(inputs["Wv_r"], inputs["Wv_i"]),
        "wor": wostack(inputs["Wo_r"], -np.asarray(inputs["Wo_i"], dtype=f32)),
        "woi": wostack(inputs["Wo_i"], inputs["Wo_r"]),
        "bqrep": brep(inputs["bq_r"], inputs["bq_i"]),
        "bkrep": brep(inputs["bk_r"], inputs["bk_i"]),
        "bvrep": brep(inputs["bv_r"], inputs["bv_i"]),
        "maskcols": mask_cols,
        "identb": np.eye(128, dtype=BF16),
    }
    m.update(xmaps[b])
    return m


def kernel(_trace=False, _trace_kwargs=None, **inputs):
    from concourse.bass_utils import run_bass_kernel_spmd

    with_bias = any(
        np.any(np.asarray(inputs[k]))
        for k in ("bq_r", "bq_i", "bk_r", "bk_i", "bv_r", "bv_i")
    )
    nc = _get_program(with_bias=bool(with_bias))
    xmaps = {b: _prep_x(inputs, b) for b in range(B)}
    in_maps = [_prep_core_inputs(inputs, c, xmaps) for c in range(NCORES)]
    res = run_bass_kernel_spmd(
        nc, in_maps, core_ids=list(range(NCORES)),
        trace=_trace, **(_trace_kwargs or {}),
    )
    _CACHE["last_results"] = res

    bo_r = np.asarray(inputs["bo_r"], dtype=np.float32)
    bo_i = np.asarray(inputs["bo_i"], dtype=np.float32)
    out = np.empty((2, B, S, F), dtype=np.float32)
    cpb = NCORES // B
    for b in range(B):
        cores = range(b * cpb, (b + 1) * cpb)
        out[0, b] = sum(res.results[c]["out_r"] for c in cores) + bo_r
        out[1, b] = sum(res.results[c]["out_i"] for c in cores) + bo_i
    return out
